# revision 19
# baseline (speedup 1.0000x reference)
"""TRN2 Bass kernel for nn_Attention_3728031613427.

GQA causal attention (B=1, S=2048, D=4096, H=32, KV=8, HD=128) with RoPE,
8-way tensor-parallel over KV heads. Each core computes 4 q-heads / 1 kv-head
and a full [D, S] partial of the output projection; partials are summed on
the host (the all-reduce of the sharding hint).

v2: scores are computed transposed (scoresT = kT.T @ qT) so the softmax
probabilities are born in the [t, s] layout AV needs -- no p transposes.
The softmax denominator l[s] falls out of the AV matmul via a ones-column
appended to V (column HD of the accumulation). All remaining 128x128
transposes (q/k/v/attn) run on the PE in transpose mode instead of the slow
DMA xbar.
"""

import sys

if "/opt/trn_rl_repo" not in sys.path:
    sys.path.insert(0, "/opt/trn_rl_repo")

from contextlib import ExitStack

import ml_dtypes
import numpy as np

import concourse.bass as bass
import concourse.mybir as mybir
import concourse.tile as tile
from concourse import bacc
from concourse.bass_utils import run_bass_kernel_spmd
from concourse.masks import make_identity

BF16 = mybir.dt.bfloat16
F32 = mybir.dt.float32
NPBF16 = ml_dtypes.bfloat16

B, S, D = 1, 2048, 4096
H, KV, HD = 32, 8, 128
NCORES = 8
QH = H // NCORES         # 4 q heads per core
SB = 128                 # s-block
TC = 512                 # s-chunk width for scoresT tiles
NSB = S // SB            # 16
NSC = S // TC            # 4
NTB = S // 128           # 16 t-blocks
DC = D // 128            # 32 contraction chunks
VW = HD + 1              # v with ones column -> l in column HD
NEG_THRESH = -1e8

_TRACE = False
_LAST_RESULTS = None


def _classify_mask(mask):
    """Classify [t-block 128, s-chunk 512] blocks of mask.T as skip / plain /
    masked, and per (s-block, t-block) AV inclusion."""
    tchunk_kind = []     # [sc][tb] -> (kind, mask_idx)
    mask_blocks = []
    for sc in range(NSC):
        row = []
        for tb in range(NTB):
            blk = mask[sc * TC:(sc + 1) * TC, tb * SB:(tb + 1) * SB]  # [s, t]
            if (blk <= NEG_THRESH).all():
                row.append(("skip", -1))
            elif (blk == 0.0).all():
                row.append(("plain", -1))
            else:
                row.append(("masked", len(mask_blocks)))
                mask_blocks.append(np.ascontiguousarray(blk.T))  # [t, s]
        tchunk_kind.append(row)
    av_tblocks = []
    for i in range(NSB):
        tbs = []
        for tb in range(NTB):
            sub = mask[i * SB:(i + 1) * SB, tb * SB:(tb + 1) * SB]
            if not (sub <= NEG_THRESH).all() and \
                    tchunk_kind[i // (TC // SB)][tb][0] != "skip":
                tbs.append(tb)
        av_tblocks.append(tbs)
    if not mask_blocks:
        mask_blocks.append(np.zeros((SB, TC), np.float32))
    return tchunk_kind, av_tblocks, np.stack(mask_blocks).astype(np.float32)


def _build_nc(tchunk_kind, av_tblocks, nmask):
    nc = bacc.Bacc()

    xT = nc.declare_dram_parameter("xT", [D, S], BF16, isOutput=False)
    wqT = nc.declare_dram_parameter("wqT", [D, QH * HD], BF16, isOutput=False)
    wkvT = nc.declare_dram_parameter("wkvT", [D, 2 * HD], BF16, isOutput=False)
    woT = nc.declare_dram_parameter("woT", [QH * HD, D], BF16, isOutput=False)
    # cos/sin tables: q tables replicated 4x across heads (scaled by
    # 1/sqrt(HD)), k tables single-head
    cq4 = nc.declare_dram_parameter("cq4", [S, QH * 64], BF16, isOutput=False)
    sq4 = nc.declare_dram_parameter("sq4", [S, QH * 64], BF16, isOutput=False)
    ck1 = nc.declare_dram_parameter("ck1", [S, 64], BF16, isOutput=False)
    sk1 = nc.declare_dram_parameter("sk1", [S, 64], BF16, isOutput=False)
    maskb = nc.declare_dram_parameter("maskb", [nmask, SB, TC], BF16, isOutput=False)
    outT = nc.declare_dram_parameter("outT", [D, S], F32, isOutput=True)

    with tile.TileContext(nc) as tc, ExitStack() as ctx:
        wpool = ctx.enter_context(tc.tile_pool(name="wpool", bufs=1))
        xpool = ctx.enter_context(tc.tile_pool(name="xpool", bufs=2))
        rpool = ctx.enter_context(tc.tile_pool(name="rpool", bufs=1))
        tpool = ctx.enter_context(tc.tile_pool(name="tpool", bufs=4))
        ppool = ctx.enter_context(tc.tile_pool(name="ppool", bufs=2))
        opool = ctx.enter_context(tc.tile_pool(name="opool", bufs=3))
        psum = ctx.enter_context(tc.tile_pool(name="psum", bufs=4, space="PSUM"))

        ident = wpool.tile([128, 128], BF16)
        make_identity(nc, ident)

        # --- resident loads ---------------------------------------------
        wq_sb = wpool.tile([128, DC, QH * HD], BF16)
        wqr = wqT.rearrange("(dc p) m -> p dc m", p=128)
        wkv_sb = wpool.tile([128, DC, 2 * HD], BF16)
        wkvr = wkvT.rearrange("(dc p) m -> p dc m", p=128)
        # o-proj weights resident, loaded off the critical queues
        wo_sb = wpool.tile([128, QH, D], BF16)
        nc.gpsimd.dma_start(wo_sb, woT.rearrange("(hb p) o -> p hb o", p=128))

        cq_sb = wpool.tile([128, NSB, QH * 64], BF16)
        nc.gpsimd.dma_start(cq_sb, cq4.rearrange("(i p) f -> p i f", p=128))
        sq_sb = wpool.tile([128, NSB, QH * 64], BF16)
        nc.gpsimd.dma_start(sq_sb, sq4.rearrange("(i p) f -> p i f", p=128))
        ck_sb = wpool.tile([128, NSB, 64], BF16)
        nc.gpsimd.dma_start(ck_sb, ck1.rearrange("(i p) f -> p i f", p=128))
        sk_sb = wpool.tile([128, NSB, 64], BF16)
        nc.gpsimd.dma_start(sk_sb, sk1.rearrange("(i p) f -> p i f", p=128))

        qrotT = rpool.tile([128, QH, S], BF16)   # [hd', h, s]
        krotT = rpool.tile([128, S], BF16)       # [hd', t]
        v_sb = rpool.tile([128, NTB, VW], BF16)  # [t-in-block, tb, d'+1]
        attnT = rpool.tile([128, QH, S], BF16)   # [d', h, s]

        def h3(ap, width):
            return ap.rearrange("p (h x) -> p h x", x=HD)[:, :, 0:width]

        def o3(ap, width):
            return ap.rearrange("p (h x) -> p h x", x=HD)[:, :, HD // 2:HD // 2 + width]

        # --- pipelined emission: for each 512-wide s-chunk, project
        # (+RoPE+transposes), then attention for all heads, then o-proj.
        # Keeps the PE dense so ACT exp hides behind matmul phases.
        XCH = 256  # x streaming chunk (doesn't affect matmul shapes)
        for xi in range(S // XCH):
            xc = xpool.tile([128, DC, XCH], BF16, tag="xc")
            xr = xT[:, xi * XCH:(xi + 1) * XCH].rearrange(
                "(dc p) s -> p dc s", p=128)
            for j in range(4):
                eng = nc.sync if (xi + j) % 2 == 0 else nc.scalar
                eng.dma_start(xc[:, j * 8:(j + 1) * 8, :],
                              xr[:, j * 8:(j + 1) * 8, :])
            if xi == 0:
                for j in range(8):
                    eng = nc.sync if j % 2 == 0 else nc.scalar
                    eng.dma_start(wq_sb[:, j * 4:(j + 1) * 4, :],
                                  wqr[:, j * 4:(j + 1) * 4, :])
                for j in range(4):
                    eng = nc.scalar if j % 2 == 0 else nc.sync
                    eng.dma_start(wkv_sb[:, j * 8:(j + 1) * 8, :],
                                  wkvr[:, j * 8:(j + 1) * 8, :])
            for ib in range(XCH // SB):
                i = xi * (XCH // SB) + ib
                sblk = slice(ib * SB, (ib + 1) * SB)
                ps_q = psum.tile([128, QH * HD], F32, tag="w512", bufs=5)
                ps_kv = psum.tile([128, 2 * HD], F32, tag="b1", bufs=3)
                for dc in range(DC):
                    nc.tensor.matmul(ps_q, xc[:, dc, sblk], wq_sb[:, dc, :],
                                     start=(dc == 0), stop=(dc == DC - 1))
                for dc in range(DC):
                    nc.tensor.matmul(ps_kv, xc[:, dc, sblk], wkv_sb[:, dc, :],
                                     start=(dc == 0), stop=(dc == DC - 1))
                # RoPE on q: all 4 heads per op via 3D APs
                qrot_n = tpool.tile([128, QH * HD], BF16, tag="qrot", bufs=2)
                c4 = cq_sb[:, i].rearrange("p (h x) -> p h x", x=64)
                s4 = sq_sb[:, i].rearrange("p (h x) -> p h x", x=64)
                t1 = tpool.tile([128, QH, 64], F32, tag="t1", bufs=2)
                t2 = tpool.tile([128, QH, 64], F32, tag="t2", bufs=2)
                nc.vector.tensor_mul(t1, h3(ps_q, 64), c4)
                nc.vector.tensor_mul(t2, o3(ps_q, 64), s4)
                nc.vector.tensor_sub(h3(qrot_n, 64), t1, t2)
                t3 = tpool.tile([128, QH, 64], F32, tag="t1", bufs=2)
                t4 = tpool.tile([128, QH, 64], F32, tag="t2", bufs=2)
                nc.vector.tensor_mul(t3, h3(ps_q, 64), s4)
                nc.vector.tensor_mul(t4, o3(ps_q, 64), c4)
                nc.vector.tensor_add(o3(qrot_n, 64), t3, t4)
                # RoPE on k; v is already [t, d']
                kv_n = tpool.tile([128, HD], BF16, tag="kv_n", bufs=2)
                e = slice(0, 64)
                o = slice(64, HD)
                t5 = tpool.tile([128, 64], F32, tag="t5", bufs=2)
                t6 = tpool.tile([128, 64], F32, tag="t6", bufs=2)
                nc.vector.tensor_mul(t5, ps_kv[:, e], ck_sb[:, i])
                nc.vector.tensor_mul(t6, ps_kv[:, o], sk_sb[:, i])
                nc.vector.tensor_sub(kv_n[:, e], t5, t6)
                t7 = tpool.tile([128, 64], F32, tag="t5", bufs=2)
                t8 = tpool.tile([128, 64], F32, tag="t6", bufs=2)
                nc.vector.tensor_mul(t7, ps_kv[:, e], sk_sb[:, i])
                nc.vector.tensor_mul(t8, ps_kv[:, o], ck_sb[:, i])
                nc.vector.tensor_add(kv_n[:, o], t7, t8)
                nc.vector.tensor_copy(v_sb[:, i, 0:HD], ps_kv[:, HD:2 * HD])
                nc.vector.memset(v_sb[:, i, HD:VW], 1.0)
                # transposes on the PE
                for h in range(QH):
                    ps_t = psum.tile([128, 128], BF16, tag="b1", bufs=3)
                    nc.tensor.transpose(ps_t, qrot_n[:, h * HD:(h + 1) * HD],
                                        ident)
                    nc.vector.tensor_copy(qrotT[:, h, i * SB:(i + 1) * SB],
                                          ps_t)
                ps_t = psum.tile([128, 128], BF16, tag="b1", bufs=3)
                nc.tensor.transpose(ps_t, kv_n, ident)
                nc.vector.tensor_copy(krotT[:, i * SB:(i + 1) * SB], ps_t)

            if xi % 2 == 0:
                continue
            sc = xi // 2
            # mask blocks for this s-chunk, shared across heads
            mtiles = {}
            for tb in range(NTB):
                kind, mi = tchunk_kind[sc][tb]
                if kind == "masked":
                    mblk = tpool.tile([128, TC], BF16, tag="mblk", bufs=4)
                    nc.gpsimd.dma_start(mblk, maskb[mi])
                    mtiles[tb] = mblk
            for h in range(QH):
                pT = ppool.tile([128, NTB, TC], BF16, tag="pT", bufs=1)
                for tb in range(NTB):
                    kind, mi = tchunk_kind[sc][tb]
                    if kind == "skip":
                        continue
                    ps_s = psum.tile([128, TC], F32, tag="w512", bufs=5)
                    nc.tensor.matmul(ps_s, krotT[:, tb * SB:(tb + 1) * SB],
                                     qrotT[:, h, sc * TC:(sc + 1) * TC],
                                     start=True, stop=True)
                    if kind == "masked":
                        nc.vector.tensor_add(ps_s, ps_s, mtiles[tb])
                    nc.scalar.activation(pT[:, tb, :], ps_s,
                                         mybir.ActivationFunctionType.Exp)
                for ib in range(TC // SB):
                    i = sc * (TC // SB) + ib
                    tbs = av_tblocks[i]
                    ps_a = psum.tile([128, VW], F32, tag="b1", bufs=3)
                    for k, tb in enumerate(tbs):
                        nc.tensor.matmul(ps_a,
                                         pT[:, tb, ib * SB:(ib + 1) * SB],
                                         v_sb[:, tb, :],
                                         start=(k == 0),
                                         stop=(k == len(tbs) - 1))
                    rl = tpool.tile([128, 1], F32, tag="rl")
                    nc.vector.reciprocal(rl, ps_a[:, HD:VW])
                    attn_n = tpool.tile([128, HD], BF16, tag="attn_n", bufs=2)
                    nc.scalar.activation(attn_n, ps_a[:, 0:HD],
                                         mybir.ActivationFunctionType.Copy,
                                         scale=rl)
                    ps_t = psum.tile([128, 128], BF16, tag="b1", bufs=3)
                    nc.tensor.transpose(ps_t, attn_n, ident)
                    nc.vector.tensor_copy(attnT[:, h, i * SB:(i + 1) * SB],
                                          ps_t)
            # o-proj for this s-chunk (resident weights)
            for ob in range(D // 128):
                ps_o = psum.tile([128, TC], F32, tag="w512", bufs=5)
                for hb in range(QH):
                    nc.tensor.matmul(
                        ps_o, wo_sb[:, hb, ob * 128:(ob + 1) * 128],
                        attnT[:, hb, sc * TC:(sc + 1) * TC],
                        start=(hb == 0), stop=(hb == QH - 1))
                osb = opool.tile([128, TC], F32, tag="osb", bufs=2)
                if ob % 2 == 0:
                    nc.scalar.copy(osb, ps_o)
                else:
                    nc.vector.tensor_copy(osb, ps_o)
                eng2 = nc.sync if ob % 2 == 1 else nc.scalar
                eng2.dma_start(
                    outT[ob * 128:(ob + 1) * 128,
                         sc * TC:(sc + 1) * TC], osb)
    nc.finalize()
    return nc


def kernel(x, wq, wk, wv, wo, cos, sin, cache, mask, start_pos):
    global _LAST_RESULTS
    x = np.asarray(x, np.float32)
    mask = np.asarray(mask, np.float32)
    cos = np.asarray(cos, np.float32)
    sin = np.asarray(sin, np.float32)

    tchunk_kind, av_tblocks, mask_blocks = _classify_mask(mask)
    nc = _build_nc(tchunk_kind, av_tblocks, mask_blocks.shape[0])

    # de-interleave permutation for the RoPE pair layout
    perm = np.concatenate([np.arange(0, HD, 2), np.arange(1, HD, 2)])
    xT = np.ascontiguousarray(x[0].T).astype(NPBF16)
    scale = np.float32(1.0 / np.sqrt(HD))
    cq = np.tile(cos * scale, (1, QH)).astype(NPBF16)   # [S, 256]
    sq = np.tile(sin * scale, (1, QH)).astype(NPBF16)
    ck = cos.astype(NPBF16)
    sk = sin.astype(NPBF16)

    in_maps = []
    for c in range(NCORES):
        wq_c = wq[c * QH * HD:(c + 1) * QH * HD].reshape(QH, HD, D)[:, perm]
        wq_c = wq_c.reshape(QH * HD, D)
        wk_c = wk[c * HD:(c + 1) * HD][perm]
        wv_c = wv[c * HD:(c + 1) * HD]
        wkv_c = np.concatenate([wk_c, wv_c], axis=0)      # [256, D]
        wo_c = wo[:, c * QH * HD:(c + 1) * QH * HD]        # [D, 512]
        in_maps.append({
            "xT": xT,
            "wqT": np.ascontiguousarray(wq_c.T).astype(NPBF16),
            "wkvT": np.ascontiguousarray(wkv_c.T).astype(NPBF16),
            "woT": np.ascontiguousarray(wo_c.T).astype(NPBF16),
            "cq4": cq, "sq4": sq, "ck1": ck, "sk1": sk,
            "maskb": mask_blocks.astype(NPBF16),
        })

    res = run_bass_kernel_spmd(nc, in_maps, core_ids=list(range(NCORES)),
                               trace=_TRACE)
    _LAST_RESULTS = res
    acc = np.zeros((D, S), np.float64)
    for r in res.results:
        acc += r["outT"].astype(np.float64)
    return acc.T.reshape(B, S, H * HD).astype(np.float32)


# revision 22
# speedup vs baseline: 1.0237x; 1.0237x over previous
"""TRN2 Bass kernel for nn_Attention_3728031613427.

GQA causal attention (B=1, S=2048, D=4096, H=32, KV=8, HD=128) with RoPE,
8-way tensor-parallel over KV heads. Each core computes 4 q-heads / 1 kv-head
and a full [D, S] partial of the output projection; partials are summed on
the host (the all-reduce of the sharding hint).

v6:
- q/k/v projections in transposed orientation (w.T stationary, x.T moving)
  so q and k are born in the [hd, s] layout the score matmuls need.
- RoPE applied in the transposed layout with packed cos/sin tables
  (cos rows 0-63, sin rows 64-127): two full-width DVE muls + two
  half-width cross-partition-window combines per head-chunk.
- scoresT = kT.T @ qT: softmax probabilities born in [t, s]; no transposes.
- softmax denominator via a ones-column appended to V (column HD of the AV
  accumulation); per-partition normalize on ACT.
- only v and attn tiles are transposed, on the PE in transpose mode.
- pipelined emission per 512-wide s-chunk: projections -> attention ->
  o-proj, so ACT exp hides behind dense PE phases.
"""

import sys

if "/opt/trn_rl_repo" not in sys.path:
    sys.path.insert(0, "/opt/trn_rl_repo")

from contextlib import ExitStack

import ml_dtypes
import numpy as np

import concourse.bass as bass
import concourse.mybir as mybir
import concourse.tile as tile
from concourse import bacc
from concourse.bass_utils import run_bass_kernel_spmd
from concourse.masks import make_identity

BF16 = mybir.dt.bfloat16
F32 = mybir.dt.float32
NPBF16 = ml_dtypes.bfloat16

B, S, D = 1, 2048, 4096
H, KV, HD = 32, 8, 128
NCORES = 8
QH = H // NCORES         # 4 q heads per core
SB = 128                 # s-block
TC = 512                 # s-chunk width
NSB = S // SB            # 16
NSC = S // TC            # 4
NTB = S // 128           # 16 t-blocks
DC = D // 128            # 32 contraction chunks
VW = HD + 1              # v with ones column -> l in column HD
NEG_THRESH = -1e8

_TRACE = False
_LAST_RESULTS = None


def _classify_mask(mask):
    """Classify [t-block 128, s-chunk 512] blocks of mask.T as skip / plain /
    masked, and per (s-block, t-block) AV inclusion."""
    tchunk_kind = []     # [sc][tb] -> (kind, mask_idx)
    mask_blocks = []
    for sc in range(NSC):
        row = []
        for tb in range(NTB):
            blk = mask[sc * TC:(sc + 1) * TC, tb * SB:(tb + 1) * SB]  # [s, t]
            if (blk <= NEG_THRESH).all():
                row.append(("skip", -1))
            elif (blk == 0.0).all():
                row.append(("plain", -1))
            else:
                row.append(("masked", len(mask_blocks)))
                mask_blocks.append(np.ascontiguousarray(blk.T))  # [t, s]
        tchunk_kind.append(row)
    av_tblocks = []
    for i in range(NSB):
        tbs = []
        for tb in range(NTB):
            sub = mask[i * SB:(i + 1) * SB, tb * SB:(tb + 1) * SB]
            if not (sub <= NEG_THRESH).all() and \
                    tchunk_kind[i // (TC // SB)][tb][0] != "skip":
                tbs.append(tb)
        av_tblocks.append(tbs)
    if not mask_blocks:
        mask_blocks.append(np.zeros((SB, TC), np.float32))
    return tchunk_kind, av_tblocks, np.stack(mask_blocks).astype(np.float32)


def _build_nc(tchunk_kind, av_tblocks, nmask):
    nc = bacc.Bacc()

    xT = nc.declare_dram_parameter("xT", [D, S], BF16, isOutput=False)
    wqT = nc.declare_dram_parameter("wqT", [D, QH * HD], BF16, isOutput=False)
    wkvT = nc.declare_dram_parameter("wkvT", [D, 2 * HD], BF16, isOutput=False)
    woT = nc.declare_dram_parameter("woT", [QH * HD, D], BF16, isOutput=False)
    # packed cos/sin tables for transposed-layout RoPE:
    # csX1 rows 0-63 = cosT, rows 64-127 = sinT; csX2 swapped.
    csq1 = nc.declare_dram_parameter("csq1", [128, S], BF16, isOutput=False)
    csq2 = nc.declare_dram_parameter("csq2", [128, S], BF16, isOutput=False)
    csk1 = nc.declare_dram_parameter("csk1", [128, S], BF16, isOutput=False)
    csk2 = nc.declare_dram_parameter("csk2", [128, S], BF16, isOutput=False)
    maskb = nc.declare_dram_parameter("maskb", [nmask, SB, TC], BF16, isOutput=False)
    outT = nc.declare_dram_parameter("outT", [D, S], F32, isOutput=True)

    with tile.TileContext(nc) as tc, ExitStack() as ctx:
        wpool = ctx.enter_context(tc.tile_pool(name="wpool", bufs=1))
        xpool = ctx.enter_context(tc.tile_pool(name="xpool", bufs=1))
        rpool = ctx.enter_context(tc.tile_pool(name="rpool", bufs=1))
        tpool = ctx.enter_context(tc.tile_pool(name="tpool", bufs=4))
        ppool = ctx.enter_context(tc.tile_pool(name="ppool", bufs=2))
        opool = ctx.enter_context(tc.tile_pool(name="opool", bufs=2))
        psum = ctx.enter_context(tc.tile_pool(name="psum", bufs=4, space="PSUM"))

        ident = wpool.tile([128, 128], BF16)
        make_identity(nc, ident)

        # --- resident loads ---------------------------------------------
        wq_sb = wpool.tile([128, DC, QH * HD], BF16)
        wqr = wqT.rearrange("(dc p) m -> p dc m", p=128)
        wkv_sb = wpool.tile([128, DC, 2 * HD], BF16)
        wkvr = wkvT.rearrange("(dc p) m -> p dc m", p=128)
        # o-proj weights resident, loaded off the critical queues
        wo_sb = wpool.tile([128, QH, D], BF16)
        nc.gpsimd.dma_start(wo_sb, woT.rearrange("(hb p) o -> p hb o", p=128))

        csq1_sb = wpool.tile([128, S], BF16)
        nc.gpsimd.dma_start(csq1_sb, csq1[:, :])
        csq2_sb = wpool.tile([128, S], BF16)
        nc.gpsimd.dma_start(csq2_sb, csq2[:, :])
        csk1_sb = wpool.tile([128, S], BF16)
        nc.gpsimd.dma_start(csk1_sb, csk1[:, :])
        csk2_sb = wpool.tile([128, S], BF16)
        nc.gpsimd.dma_start(csk2_sb, csk2[:, :])

        qrotT = rpool.tile([128, QH, S], BF16)   # [hd', h, s]
        krotT = rpool.tile([128, S], BF16)       # [hd', t]
        v_sb = rpool.tile([128, NTB, VW], BF16)  # [t-in-block, tb, d'+1]
        attnT = rpool.tile([128, QH, S], BF16)   # [d', h, s]

        for sc in range(NSC):
            ch = slice(sc * TC, (sc + 1) * TC)
            # ---- projections (transposed orientation) + RoPE ----------
            xc = xpool.tile([128, DC, TC], BF16, tag="xc")
            xr = xT[:, ch].rearrange("(dc p) s -> p dc s", p=128)
            for j in range(4):
                eng = nc.sync if (sc + j) % 2 == 0 else nc.scalar
                eng.dma_start(xc[:, j * 8:(j + 1) * 8, :],
                              xr[:, j * 8:(j + 1) * 8, :])
            if sc == 0:
                for j in range(8):
                    eng = nc.sync if j % 2 == 0 else nc.scalar
                    eng.dma_start(wq_sb[:, j * 4:(j + 1) * 4, :],
                                  wqr[:, j * 4:(j + 1) * 4, :])
                for j in range(4):
                    eng = nc.scalar if j % 2 == 0 else nc.sync
                    eng.dma_start(wkv_sb[:, j * 8:(j + 1) * 8, :],
                                  wkvr[:, j * 8:(j + 1) * 8, :])

            def rope_T(ps, out_ap, tab1, tab2):
                # every op's two inputs share a base partition; the
                # cross-half data movement happens on the write side
                ta = tpool.tile([128, TC], F32, tag="tc", bufs=2)
                tb = tpool.tile([128, TC], F32, tag="ts", bufs=2)
                lo, hi = slice(0, 64), slice(64, 128)
                nc.vector.tensor_mul(ta[lo], ps[lo], tab1[lo, ch])    # qe*c
                nc.vector.tensor_mul(tb[lo], ps[hi], tab1[hi, ch])    # qo*s
                nc.vector.tensor_sub(out_ap[lo], ta[lo], tb[lo])
                nc.vector.tensor_mul(ta[hi], ps[lo], tab2[lo, ch])    # qe*s
                nc.vector.tensor_mul(tb[hi], ps[hi], tab2[hi, ch])    # qo*c
                nc.vector.tensor_add(out_ap[hi], ta[hi], tb[hi])

            for h in range(QH):
                ps_q = psum.tile([128, TC], F32, tag="w512", bufs=5)
                for dc in range(DC):
                    nc.tensor.matmul(ps_q,
                                     wq_sb[:, dc, h * HD:(h + 1) * HD],
                                     xc[:, dc, :],
                                     start=(dc == 0), stop=(dc == DC - 1))
                rope_T(ps_q, qrotT[:, h, ch], csq1_sb, csq2_sb)
            ps_k = psum.tile([128, TC], F32, tag="w512", bufs=5)
            for dc in range(DC):
                nc.tensor.matmul(ps_k, wkv_sb[:, dc, 0:HD], xc[:, dc, :],
                                 start=(dc == 0), stop=(dc == DC - 1))
            rope_T(ps_k, krotT[:, ch], csk1_sb, csk2_sb)
            ps_v = psum.tile([128, TC], F32, tag="w512", bufs=5)
            for dc in range(DC):
                nc.tensor.matmul(ps_v, wkv_sb[:, dc, HD:2 * HD], xc[:, dc, :],
                                 start=(dc == 0), stop=(dc == DC - 1))
            vT_n = tpool.tile([128, TC], BF16, tag="vT_n", bufs=2)
            nc.vector.tensor_copy(vT_n, ps_v)
            for j in range(TC // SB):
                tb = sc * (TC // SB) + j
                ps_t = psum.tile([128, 128], BF16, tag="b1", bufs=3)
                nc.tensor.transpose(ps_t, vT_n[:, j * SB:(j + 1) * SB], ident)
                nc.vector.tensor_copy(v_sb[:, tb, 0:HD], ps_t)
                nc.vector.memset(v_sb[:, tb, HD:VW], 1.0)

            # ---- attention for this s-chunk ---------------------------
            mtiles = {}
            for tb in range(NTB):
                kind, mi = tchunk_kind[sc][tb]
                if kind == "masked":
                    mblk = tpool.tile([128, TC], BF16, tag="mblk", bufs=4)
                    nc.gpsimd.dma_start(mblk, maskb[mi])
                    mtiles[tb] = mblk
            for h in range(QH):
                pT = ppool.tile([128, NTB, TC], BF16, tag="pT", bufs=1)
                for tb in range(NTB):
                    kind, mi = tchunk_kind[sc][tb]
                    if kind == "skip":
                        continue
                    ps_s = psum.tile([128, TC], F32, tag="w512", bufs=5)
                    nc.tensor.matmul(ps_s, krotT[:, tb * SB:(tb + 1) * SB],
                                     qrotT[:, h, ch],
                                     start=True, stop=True)
                    if kind == "masked":
                        nc.vector.tensor_add(ps_s, ps_s, mtiles[tb])
                    nc.scalar.activation(pT[:, tb, :], ps_s,
                                         mybir.ActivationFunctionType.Exp)
                for ib in range(TC // SB):
                    i = sc * (TC // SB) + ib
                    tbs = av_tblocks[i]
                    ps_a = psum.tile([128, VW], F32, tag="b1", bufs=3)
                    for k, tb in enumerate(tbs):
                        nc.tensor.matmul(ps_a,
                                         pT[:, tb, ib * SB:(ib + 1) * SB],
                                         v_sb[:, tb, :],
                                         start=(k == 0),
                                         stop=(k == len(tbs) - 1))
                    rl = tpool.tile([128, 1], F32, tag="rl")
                    nc.vector.reciprocal(rl, ps_a[:, HD:VW])
                    attn_n = tpool.tile([128, HD], BF16, tag="attn_n", bufs=2)
                    nc.scalar.activation(attn_n, ps_a[:, 0:HD],
                                         mybir.ActivationFunctionType.Copy,
                                         scale=rl)
                    ps_t = psum.tile([128, 128], BF16, tag="b1", bufs=3)
                    nc.tensor.transpose(ps_t, attn_n, ident)
                    nc.vector.tensor_copy(attnT[:, h, i * SB:(i + 1) * SB],
                                          ps_t)

            # ---- o-proj for this s-chunk (resident weights) -----------
            for ob in range(D // 128):
                ps_o = psum.tile([128, TC], F32, tag="w512", bufs=5)
                for hb in range(QH):
                    nc.tensor.matmul(
                        ps_o, wo_sb[:, hb, ob * 128:(ob + 1) * 128],
                        attnT[:, hb, ch],
                        start=(hb == 0), stop=(hb == QH - 1))
                osb = opool.tile([128, TC], F32, tag="osb", bufs=2)
                if ob % 2 == 0:
                    nc.scalar.copy(osb, ps_o)
                else:
                    nc.vector.tensor_copy(osb, ps_o)
                eng2 = nc.sync if ob % 2 == 1 else nc.scalar
                eng2.dma_start(
                    outT[ob * 128:(ob + 1) * 128, ch], osb)
    nc.finalize()
    return nc


def kernel(x, wq, wk, wv, wo, cos, sin, cache, mask, start_pos):
    global _LAST_RESULTS
    x = np.asarray(x, np.float32)
    mask = np.asarray(mask, np.float32)
    cos = np.asarray(cos, np.float32)
    sin = np.asarray(sin, np.float32)

    tchunk_kind, av_tblocks, mask_blocks = _classify_mask(mask)
    nc = _build_nc(tchunk_kind, av_tblocks, mask_blocks.shape[0])

    # de-interleave permutation for the RoPE pair layout
    perm = np.concatenate([np.arange(0, HD, 2), np.arange(1, HD, 2)])
    xT = np.ascontiguousarray(x[0].T).astype(NPBF16)
    scale = np.float32(1.0 / np.sqrt(HD))
    csq1 = np.concatenate([cos.T * scale, sin.T * scale]).astype(NPBF16)
    csq2 = np.concatenate([sin.T * scale, cos.T * scale]).astype(NPBF16)
    csk1 = np.concatenate([cos.T, sin.T]).astype(NPBF16)
    csk2 = np.concatenate([sin.T, cos.T]).astype(NPBF16)

    in_maps = []
    for c in range(NCORES):
        wq_c = wq[c * QH * HD:(c + 1) * QH * HD].reshape(QH, HD, D)[:, perm]
        wq_c = wq_c.reshape(QH * HD, D)
        wk_c = wk[c * HD:(c + 1) * HD][perm]
        wv_c = wv[c * HD:(c + 1) * HD]
        wkv_c = np.concatenate([wk_c, wv_c], axis=0)      # [256, D]
        wo_c = wo[:, c * QH * HD:(c + 1) * QH * HD]        # [D, 512]
        in_maps.append({
            "xT": xT,
            "wqT": np.ascontiguousarray(wq_c.T).astype(NPBF16),
            "wkvT": np.ascontiguousarray(wkv_c.T).astype(NPBF16),
            "woT": np.ascontiguousarray(wo_c.T).astype(NPBF16),
            "csq1": csq1, "csq2": csq2, "csk1": csk1, "csk2": csk2,
            "maskb": mask_blocks.astype(NPBF16),
        })

    res = run_bass_kernel_spmd(nc, in_maps, core_ids=list(range(NCORES)),
                               trace=_TRACE)
    _LAST_RESULTS = res
    acc = np.zeros((D, S), np.float64)
    for r in res.results:
        acc += r["outT"].astype(np.float64)
    return acc.T.reshape(B, S, H * HD).astype(np.float32)


# revision 23
# speedup vs baseline: 1.1664x; 1.1394x over previous
"""TRN2 Bass kernel for nn_Attention_3728031613427.

GQA causal attention (B=1, S=2048, D=4096, H=32, KV=8, HD=128) with RoPE,
8-way tensor-parallel over KV heads. Each core computes 4 q-heads / 1 kv-head
and a full [D, S] partial of the output projection; partials are summed on
the host (the all-reduce of the sharding hint).

v6:
- q/k/v projections in transposed orientation (w.T stationary, x.T moving)
  so q and k are born in the [hd, s] layout the score matmuls need.
- RoPE applied in the transposed layout with packed cos/sin tables
  (cos rows 0-63, sin rows 64-127): two full-width DVE muls + two
  half-width cross-partition-window combines per head-chunk.
- scoresT = kT.T @ qT: softmax probabilities born in [t, s]; no transposes.
- softmax denominator via a ones-column appended to V (column HD of the AV
  accumulation); per-partition normalize on ACT.
- only v and attn tiles are transposed, on the PE in transpose mode.
- pipelined emission per 512-wide s-chunk: projections -> attention ->
  o-proj, so ACT exp hides behind dense PE phases.
"""

import sys

if "/opt/trn_rl_repo" not in sys.path:
    sys.path.insert(0, "/opt/trn_rl_repo")

from contextlib import ExitStack

import ml_dtypes
import numpy as np

import concourse.bass as bass
import concourse.mybir as mybir
import concourse.tile as tile
from concourse import bacc
from concourse.bass_utils import run_bass_kernel_spmd
from concourse.masks import make_identity

BF16 = mybir.dt.bfloat16
F32 = mybir.dt.float32
NPBF16 = ml_dtypes.bfloat16

B, S, D = 1, 2048, 4096
H, KV, HD = 32, 8, 128
NCORES = 8
QH = H // NCORES         # 4 q heads per core
SB = 128                 # s-block
TC = 512                 # s-chunk width
NSB = S // SB            # 16
NSC = S // TC            # 4
NTB = S // 128           # 16 t-blocks
DC = D // 128            # 32 contraction chunks
VW = HD + 1              # v with ones column -> l in column HD
NEG_THRESH = -1e8

_TRACE = False
_LAST_RESULTS = None


def _classify_mask(mask):
    """Classify [t-block 128, s-chunk 512] blocks of mask.T as skip / plain /
    masked, and per (s-block, t-block) AV inclusion."""
    tchunk_kind = []     # [sc][tb] -> (kind, mask_idx)
    mask_blocks = []
    for sc in range(NSC):
        row = []
        for tb in range(NTB):
            blk = mask[sc * TC:(sc + 1) * TC, tb * SB:(tb + 1) * SB]  # [s, t]
            if (blk <= NEG_THRESH).all():
                row.append(("skip", -1))
            elif (blk == 0.0).all():
                row.append(("plain", -1))
            else:
                row.append(("masked", len(mask_blocks)))
                mask_blocks.append(np.ascontiguousarray(blk.T))  # [t, s]
        tchunk_kind.append(row)
    av_tblocks = []
    for i in range(NSB):
        tbs = []
        for tb in range(NTB):
            sub = mask[i * SB:(i + 1) * SB, tb * SB:(tb + 1) * SB]
            if not (sub <= NEG_THRESH).all() and \
                    tchunk_kind[i // (TC // SB)][tb][0] != "skip":
                tbs.append(tb)
        av_tblocks.append(tbs)
    if not mask_blocks:
        mask_blocks.append(np.zeros((SB, TC), np.float32))
    return tchunk_kind, av_tblocks, np.stack(mask_blocks).astype(np.float32)


def _build_nc(tchunk_kind, av_tblocks, nmask):
    nc = bacc.Bacc()

    xT = nc.declare_dram_parameter("xT", [D, S], BF16, isOutput=False)
    wqT = nc.declare_dram_parameter("wqT", [D, QH * HD], BF16, isOutput=False)
    wkvT = nc.declare_dram_parameter("wkvT", [D, 2 * HD], BF16, isOutput=False)
    woT = nc.declare_dram_parameter("woT", [QH * HD, D], BF16, isOutput=False)
    # packed cos/sin tables for transposed-layout RoPE:
    # csX1 rows 0-63 = cosT, rows 64-127 = sinT; csX2 swapped.
    csq1 = nc.declare_dram_parameter("csq1", [128, S], BF16, isOutput=False)
    csq2 = nc.declare_dram_parameter("csq2", [128, S], BF16, isOutput=False)
    csk1 = nc.declare_dram_parameter("csk1", [128, S], BF16, isOutput=False)
    csk2 = nc.declare_dram_parameter("csk2", [128, S], BF16, isOutput=False)
    maskb = nc.declare_dram_parameter("maskb", [nmask, SB, TC], BF16, isOutput=False)
    outT = nc.declare_dram_parameter("outT", [D, S], F32, isOutput=True)

    with tile.TileContext(nc) as tc, ExitStack() as ctx:
        wpool = ctx.enter_context(tc.tile_pool(name="wpool", bufs=1))
        xpool = ctx.enter_context(tc.tile_pool(name="xpool", bufs=1))
        rpool = ctx.enter_context(tc.tile_pool(name="rpool", bufs=1))
        tpool = ctx.enter_context(tc.tile_pool(name="tpool", bufs=4))
        ppool = ctx.enter_context(tc.tile_pool(name="ppool", bufs=2))
        opool = ctx.enter_context(tc.tile_pool(name="opool", bufs=2))
        psum = ctx.enter_context(tc.tile_pool(name="psum", bufs=4, space="PSUM"))

        ident = wpool.tile([128, 128], BF16)
        make_identity(nc, ident)

        # --- resident loads ---------------------------------------------
        wq_sb = wpool.tile([128, DC, QH * HD], BF16)
        wqr = wqT.rearrange("(dc p) m -> p dc m", p=128)
        wkv_sb = wpool.tile([128, DC, 2 * HD], BF16)
        wkvr = wkvT.rearrange("(dc p) m -> p dc m", p=128)
        # o-proj weights resident, loaded off the critical queues
        wo_sb = wpool.tile([128, QH, D], BF16)
        nc.gpsimd.dma_start(wo_sb, woT.rearrange("(hb p) o -> p hb o", p=128))

        csq1_sb = wpool.tile([128, S], BF16)
        nc.gpsimd.dma_start(csq1_sb, csq1[:, :])
        csq2_sb = wpool.tile([128, S], BF16)
        nc.gpsimd.dma_start(csq2_sb, csq2[:, :])
        csk1_sb = wpool.tile([128, S], BF16)
        nc.gpsimd.dma_start(csk1_sb, csk1[:, :])
        csk2_sb = wpool.tile([128, S], BF16)
        nc.gpsimd.dma_start(csk2_sb, csk2[:, :])

        krotT = rpool.tile([128, S], BF16)       # [hd', t]
        v_sb = rpool.tile([128, NTB, VW], BF16)  # [t-in-block, tb, d'+1]

        for sc in range(NSC):
            ch = slice(sc * TC, (sc + 1) * TC)
            qrotT = ppool.tile([128, QH, TC], BF16, tag="qrotT", bufs=2)
            attnT = ppool.tile([128, QH, TC], BF16, tag="attnT", bufs=2)
            # ---- projections (transposed orientation) + RoPE ----------
            xc = xpool.tile([128, DC, TC], BF16, tag="xc")
            xr = xT[:, ch].rearrange("(dc p) s -> p dc s", p=128)
            for j in range(4):
                eng = nc.sync if (sc + j) % 2 == 0 else nc.scalar
                eng.dma_start(xc[:, j * 8:(j + 1) * 8, :],
                              xr[:, j * 8:(j + 1) * 8, :])
            if sc == 0:
                for j in range(8):
                    eng = nc.sync if j % 2 == 0 else nc.scalar
                    eng.dma_start(wq_sb[:, j * 4:(j + 1) * 4, :],
                                  wqr[:, j * 4:(j + 1) * 4, :])
                for j in range(4):
                    eng = nc.scalar if j % 2 == 0 else nc.sync
                    eng.dma_start(wkv_sb[:, j * 8:(j + 1) * 8, :],
                                  wkvr[:, j * 8:(j + 1) * 8, :])

            def rope_T(ps, out_ap, tab1, tab2):
                # every op's two inputs share a base partition; the
                # cross-half data movement happens on the write side
                ta = tpool.tile([128, TC], F32, tag="tc", bufs=2)
                tb = tpool.tile([128, TC], F32, tag="ts", bufs=2)
                lo, hi = slice(0, 64), slice(64, 128)
                nc.vector.tensor_mul(ta[lo], ps[lo], tab1[lo, ch])    # qe*c
                nc.vector.tensor_mul(tb[lo], ps[hi], tab1[hi, ch])    # qo*s
                nc.vector.tensor_sub(out_ap[lo], ta[lo], tb[lo])
                nc.vector.tensor_mul(ta[hi], ps[lo], tab2[lo, ch])    # qe*s
                nc.vector.tensor_mul(tb[hi], ps[hi], tab2[hi, ch])    # qo*c
                nc.vector.tensor_add(out_ap[hi], ta[hi], tb[hi])

            for h in range(QH):
                ps_q = psum.tile([128, TC], F32, tag="w512", bufs=4)
                for dc in range(DC):
                    nc.tensor.matmul(ps_q,
                                     wq_sb[:, dc, h * HD:(h + 1) * HD],
                                     xc[:, dc, :],
                                     start=(dc == 0), stop=(dc == DC - 1))
                rope_T(ps_q, qrotT[:, h, :], csq1_sb, csq2_sb)
            ps_k = psum.tile([128, TC], F32, tag="w512", bufs=4)
            for dc in range(DC):
                nc.tensor.matmul(ps_k, wkv_sb[:, dc, 0:HD], xc[:, dc, :],
                                 start=(dc == 0), stop=(dc == DC - 1))
            rope_T(ps_k, krotT[:, ch], csk1_sb, csk2_sb)
            ps_v = psum.tile([128, TC], F32, tag="w512", bufs=4)
            for dc in range(DC):
                nc.tensor.matmul(ps_v, wkv_sb[:, dc, HD:2 * HD], xc[:, dc, :],
                                 start=(dc == 0), stop=(dc == DC - 1))
            vT_n = tpool.tile([128, TC], BF16, tag="vT_n", bufs=2)
            nc.vector.tensor_copy(vT_n, ps_v)
            for j in range(TC // SB):
                tb = sc * (TC // SB) + j
                ps_t = psum.tile([128, 128], BF16, tag="b1", bufs=2)
                nc.tensor.transpose(ps_t, vT_n[:, j * SB:(j + 1) * SB], ident)
                nc.vector.tensor_copy(v_sb[:, tb, 0:HD], ps_t)
                nc.vector.memset(v_sb[:, tb, HD:VW], 1.0)

            # ---- attention for this s-chunk ---------------------------
            mtiles = {}
            for tb in range(NTB):
                kind, mi = tchunk_kind[sc][tb]
                if kind == "masked":
                    mblk = tpool.tile([128, TC], BF16, tag="mblk", bufs=4)
                    nc.gpsimd.dma_start(mblk, maskb[mi])
                    mtiles[tb] = mblk
            for h in range(QH):
                pT = ppool.tile([128, NTB, TC], BF16, tag="pT", bufs=2)
                for tb in range(NTB):
                    kind, mi = tchunk_kind[sc][tb]
                    if kind == "skip":
                        continue
                    ps_s = psum.tile([128, TC], F32, tag="w512", bufs=4)
                    nc.tensor.matmul(ps_s, krotT[:, tb * SB:(tb + 1) * SB],
                                     qrotT[:, h, :],
                                     start=True, stop=True)
                    if kind == "masked":
                        nc.vector.tensor_add(ps_s, ps_s, mtiles[tb])
                    nc.scalar.activation(pT[:, tb, :], ps_s,
                                         mybir.ActivationFunctionType.Exp)
                for ib in range(TC // SB):
                    i = sc * (TC // SB) + ib
                    tbs = av_tblocks[i]
                    ps_a = psum.tile([128, VW], F32, tag="b1", bufs=2)
                    for k, tb in enumerate(tbs):
                        nc.tensor.matmul(ps_a,
                                         pT[:, tb, ib * SB:(ib + 1) * SB],
                                         v_sb[:, tb, :],
                                         start=(k == 0),
                                         stop=(k == len(tbs) - 1))
                    rl = tpool.tile([128, 1], F32, tag="rl")
                    nc.vector.reciprocal(rl, ps_a[:, HD:VW])
                    attn_n = tpool.tile([128, HD], BF16, tag="attn_n", bufs=2)
                    nc.scalar.activation(attn_n, ps_a[:, 0:HD],
                                         mybir.ActivationFunctionType.Copy,
                                         scale=rl)
                    ps_t = psum.tile([128, 128], BF16, tag="b1", bufs=2)
                    nc.tensor.transpose(ps_t, attn_n, ident)
                    nc.vector.tensor_copy(attnT[:, h, ib * SB:(ib + 1) * SB],
                                          ps_t)

            # ---- o-proj for this s-chunk (resident weights) -----------
            for ob in range(D // 128):
                ps_o = psum.tile([128, TC], F32, tag="o512", bufs=2)
                for hb in range(QH):
                    nc.tensor.matmul(
                        ps_o, wo_sb[:, hb, ob * 128:(ob + 1) * 128],
                        attnT[:, hb, :],
                        start=(hb == 0), stop=(hb == QH - 1))
                osb = opool.tile([128, TC], F32, tag="osb", bufs=2)
                if ob % 2 == 0:
                    nc.scalar.copy(osb, ps_o)
                else:
                    nc.vector.tensor_copy(osb, ps_o)
                eng2 = nc.sync if ob % 2 == 1 else nc.scalar
                eng2.dma_start(
                    outT[ob * 128:(ob + 1) * 128, ch], osb)
    nc.finalize()
    return nc


def kernel(x, wq, wk, wv, wo, cos, sin, cache, mask, start_pos):
    global _LAST_RESULTS
    x = np.asarray(x, np.float32)
    mask = np.asarray(mask, np.float32)
    cos = np.asarray(cos, np.float32)
    sin = np.asarray(sin, np.float32)

    tchunk_kind, av_tblocks, mask_blocks = _classify_mask(mask)
    nc = _build_nc(tchunk_kind, av_tblocks, mask_blocks.shape[0])

    # de-interleave permutation for the RoPE pair layout
    perm = np.concatenate([np.arange(0, HD, 2), np.arange(1, HD, 2)])
    xT = np.ascontiguousarray(x[0].T).astype(NPBF16)
    scale = np.float32(1.0 / np.sqrt(HD))
    csq1 = np.concatenate([cos.T * scale, sin.T * scale]).astype(NPBF16)
    csq2 = np.concatenate([sin.T * scale, cos.T * scale]).astype(NPBF16)
    csk1 = np.concatenate([cos.T, sin.T]).astype(NPBF16)
    csk2 = np.concatenate([sin.T, cos.T]).astype(NPBF16)

    in_maps = []
    for c in range(NCORES):
        wq_c = wq[c * QH * HD:(c + 1) * QH * HD].reshape(QH, HD, D)[:, perm]
        wq_c = wq_c.reshape(QH * HD, D)
        wk_c = wk[c * HD:(c + 1) * HD][perm]
        wv_c = wv[c * HD:(c + 1) * HD]
        wkv_c = np.concatenate([wk_c, wv_c], axis=0)      # [256, D]
        wo_c = wo[:, c * QH * HD:(c + 1) * QH * HD]        # [D, 512]
        in_maps.append({
            "xT": xT,
            "wqT": np.ascontiguousarray(wq_c.T).astype(NPBF16),
            "wkvT": np.ascontiguousarray(wkv_c.T).astype(NPBF16),
            "woT": np.ascontiguousarray(wo_c.T).astype(NPBF16),
            "csq1": csq1, "csq2": csq2, "csk1": csk1, "csk2": csk2,
            "maskb": mask_blocks.astype(NPBF16),
        })

    res = run_bass_kernel_spmd(nc, in_maps, core_ids=list(range(NCORES)),
                               trace=_TRACE)
    _LAST_RESULTS = res
    acc = np.zeros((D, S), np.float64)
    for r in res.results:
        acc += r["outT"].astype(np.float64)
    return acc.T.reshape(B, S, H * HD).astype(np.float32)
